# revision 24
# baseline (speedup 1.0000x reference)
"""Bahdanau attention decoder step on 8 Trainium2 NeuronCores.

Sharding:
  - LSTM cell: hidden-sharded (each core computes 128 of the 1024 hidden units
    for the full batch; reads only its slice of W_ih/W_hh).
  - Attention: batch-sharded (each core owns 8 of the 64 batch rows and its
    slice of encoder_outputs, read once into SBUF and reused for keys + ctx).
  - Output projection + embedding table: vocab-sharded columns of out_w.
  - Two small AllGathers stitch the shardings together (h1^T after the LSTM,
    input_feed_new^T before the logits matmul).
All matmuls run as fp32r (full-rate fp32 path on the PE).
"""
import numpy as np
from contextlib import ExitStack

import concourse.bass as bass
import concourse.tile as tile
from concourse import bacc, mybir
from concourse.bass import ds
from concourse.bass_utils import run_bass_kernel_spmd

N_CORES = 8
B, S, E, H, A, V = 64, 400, 512, 1024, 512, 50257
P = 128
BSL = B // N_CORES            # 8 batch rows per core
HSL = H // N_CORES            # 128 hidden units per core
VP = -(-V // N_CORES)         # 6283 vocab rows per core (last core ragged)
VPAD = 13 * 512               # 6656, padded per-core vocab width
NKC = H // P                  # 8 contraction chunks of 128 over H
F32 = mybir.dt.float32
F32R = mybir.dt.float32r


def r(ap):
    return ap.bitcast(F32R)


def build():
    nc = bacc.Bacc("TRN2", target_bir_lowering=False, debug=False,
                   num_devices=N_CORES, enable_partition_id=True)

    # ---- I/O ----
    inp = {}
    def di(name, shape, dtype=F32):
        inp[name] = nc.dram_tensor(name, list(shape), dtype, kind="ExternalInput").ap()
        return inp[name]

    embT = di("embT", (E, B))            # emb_table[tokens]^T (host gather: 64 rows)
    xT = di("xT", (E + H, B))            # only the input_feed part is used
    h0T = di("h0T", (H, B))
    c0_c = di("c0_c", (B, HSL))
    w_ihT = di("w_ihT", (E + H, 4 * HSL))
    w_hhT = di("w_hhT", (H, 4 * HSL))
    bias_c = di("bias_c", (1, 4 * HSL))
    attnq_wT = di("attnq_wT", (H, A))
    attnq_b = di("attnq_b", (A // P, P))
    attnm_wT = di("attnm_wT", (H, A))
    align_wT = di("align_wT", (A // P, P))
    encT = di("encT", (H, BSL, S))
    lin_out_wT = di("lin_out_wT", (2 * H, H))
    lin_out_b = di("lin_out_b", (1, H))
    out_wT = di("out_wT", (H, VPAD))
    out_b_c = di("out_b_c", (1, VPAD))

    logits_o = nc.dram_tensor("logits_o", [B, VPAD], F32, kind="ExternalOutput").ap()
    h1_o = nc.dram_tensor("h1_o", [B, HSL], F32, kind="ExternalOutput").ap()
    c1_o = nc.dram_tensor("c1_o", [B, HSL], F32, kind="ExternalOutput").ap()
    attn_o = nc.dram_tensor("attn_o", [BSL, S], F32, kind="ExternalOutput").ap()
    ifn_o = nc.dram_tensor("ifn_o", [BSL, H], F32, kind="ExternalOutput").ap()

    from concourse.masks import make_identity

    with tile.TileContext(nc) as tc, nc.allow_low_precision(reason="f32r operand tiles"):
        with ExitStack() as ctx:
            const = ctx.enter_context(tc.tile_pool(name="const", bufs=1))
            small = ctx.enter_context(tc.tile_pool(name="small", bufs=1))
            wstream = ctx.enter_context(tc.tile_pool(name="wstream", bufs=3))
            owpool = ctx.enter_context(tc.tile_pool(name="owpool", bufs=7))
            encpool = ctx.enter_context(tc.tile_pool(name="encpool", bufs=1))
            attw = ctx.enter_context(tc.tile_pool(name="attw", bufs=1))
            tanhp = ctx.enter_context(tc.tile_pool(name="tanhp", bufs=5))
            scr = ctx.enter_context(tc.tile_pool(name="scr", bufs=2))
            # PSUM: 8 banks total -> keys 4 + m 2 + t 2
            ps_keys = ctx.enter_context(tc.tile_pool(name="ps_keys", bufs=4, space="PSUM"))
            ps_m = ctx.enter_context(tc.tile_pool(name="ps_m", bufs=2, space="PSUM"))
            ps_t = ctx.enter_context(tc.tile_pool(name="ps_t", bufs=2, space="PSUM"))
            dram = ctx.enter_context(tc.tile_pool(name="dram", bufs=1, space="DRAM"))

            def ps_m_tile(shape):
                return ps_m.tile(shape, F32, tag="m", name="psm")

            def ps_t_tile(shape):
                return ps_t.tile(shape, F32, tag="t", name="pst")

            ident = const.tile([P, P], F32)
            make_identity(nc, ident)
            ones = const.tile([1, P], F32R)
            nc.vector.memset(ones[:].bitcast(F32), 1.0)

            # xTs holds x^T chunks: 0..3 emb^T, 4..11 input_feed^T
            xTs = const.tile([P, 12, B], F32R)
            nc.sync.dma_start(xTs[:, 0:4, :], r(embT[:].rearrange("(k p) b -> p k b", p=P)))
            nc.sync.dma_start(xTs[:, 4:12, :], r(xT[E:, :].rearrange("(k p) b -> p k b", p=P)))
            h0Ts = const.tile([P, NKC, B], F32R)
            nc.sync.dma_start(h0Ts[:], r(h0T[:].rearrange("(k p) b -> p k b", p=P)))

            # ---------- LSTM gates ----------
            gates = ps_m_tile([B, 4 * HSL])
            wg = []
            for k in range(12):
                wt = wstream.tile([P, 4 * HSL], F32R, tag="w", name="wt")
                nc.sync.dma_start(wt[:], r(w_ihT[bass.ts(k, P), :]))
                wg.append((xTs[:, k, :], wt))
            for k in range(NKC):
                wt = wstream.tile([P, 4 * HSL], F32R, tag="w", name="wt")
                nc.sync.dma_start(wt[:], r(w_hhT[bass.ts(k, P), :]))
                wg.append((h0Ts[:, k, :], wt))
            for i, (lhsT, rhs) in enumerate(wg):
                nc.tensor.matmul(gates[:], r(lhsT), r(rhs[:]), start=(i == 0), stop=False)
            bias_t = small.tile([1, 4 * HSL], F32R)
            nc.sync.dma_start(bias_t[:], r(bias_c[:]))
            nc.tensor.matmul(gates[:], r(ones[:1, :B]), r(bias_t[:]), start=False, stop=True)

            # ---------- LSTM nonlinearities ----------
            sig_i = small.tile([B, HSL], F32)
            sig_f = small.tile([B, HSL], F32)
            tanh_g = small.tile([B, HSL], F32)
            sig_o = small.tile([B, HSL], F32)
            AF = mybir.ActivationFunctionType
            nc.scalar.activation(sig_i[:], gates[:, 0 * HSL:1 * HSL], AF.Sigmoid)
            nc.scalar.activation(sig_f[:], gates[:, 1 * HSL:2 * HSL], AF.Sigmoid)
            nc.scalar.activation(tanh_g[:], gates[:, 2 * HSL:3 * HSL], AF.Tanh)
            nc.scalar.activation(sig_o[:], gates[:, 3 * HSL:4 * HSL], AF.Sigmoid)
            c0_t = small.tile([B, HSL], F32)
            nc.sync.dma_start(c0_t[:], c0_c[:])
            c1 = small.tile([B, HSL], F32)
            nc.vector.tensor_mul(c1[:], sig_f[:], c0_t[:])
            ig = small.tile([B, HSL], F32)
            nc.vector.tensor_mul(ig[:], sig_i[:], tanh_g[:])
            nc.vector.tensor_add(c1[:], c1[:], ig[:])
            nc.sync.dma_start(c1_o[:], c1[:])
            tc1 = small.tile([B, HSL], F32)
            nc.scalar.activation(tc1[:], c1[:], AF.Tanh)
            h1c = small.tile([B, HSL], F32)
            nc.vector.tensor_mul(h1c[:], sig_o[:], tc1[:])
            nc.sync.dma_start(h1_o[:], h1c[:])

            # h1^T slice and AllGather
            pt = ps_t_tile([P, B])
            nc.tensor.transpose(pt[:], h1c[:], ident[:B, :B])
            h1t = small.tile([P, B], F32)
            nc.vector.tensor_copy(h1t[:], pt[:])
            # AllToAll: shard j of our h1t (batch cols of receiver j) -> slot rank on core j.
            # Receiver ends with slot i = h1^T[i-th hidden slice, own batch cols]: static slices.
            ag1_in = dram.tile([N_CORES, P, BSL], F32)
            ag1_out = dram.tile([N_CORES, P, BSL], F32)
            nc.sync.dma_start(ag1_in[:].rearrange("c p b -> p c b"), h1t[:])
            nc.gpsimd.collective_compute(
                "AllToAll", mybir.AluOpType.bypass,
                replica_groups=[list(range(N_CORES))],
                ins=[ag1_in.opt()], outs=[ag1_out.opt()],
            )
            h1T_b = const.tile([P, NKC, BSL], F32R)
            nc.sync.dma_start(h1T_b[:], r(ag1_out[:].rearrange("c p b -> p c b")))

            # ---------- query^T = attnq_w @ h1_b^T + attnq_b ----------
            aqw = attw.tile([P, NKC, A], F32R, tag="aw", name="aqw")
            nc.sync.dma_start(aqw[:], r(attnq_wT[:].rearrange("(k p) a -> p k a", p=P)))
            aqb = const.tile([P, A // P], F32)
            nc.sync.dma_start(aqb[:], attnq_b[:].rearrange("a p -> p a"))
            queryT = const.tile([P, A // P, BSL], F32)
            for a in range(A // P):
                pq = ps_m_tile([P, BSL])
                for k in range(NKC):
                    nc.tensor.matmul(pq[:], r(aqw[:, k, bass.ts(a, P)]), r(h1T_b[:, k, :]),
                                     start=(k == 0), stop=(k == NKC - 1))
                nc.scalar.activation(queryT[:, a, :], pq[:], AF.Identity, bias=aqb[:, a:a + 1])

            # ---------- encoder slice resident + keys/tanh/scores ----------
            enc_s = encpool.tile([P, NKC, BSL, S], F32R)
            nc.sync.dma_start(enc_s[:], r(encT[:].rearrange("(k p) b s -> p k b s", p=P)))
            amw = attw.tile([P, NKC, A], F32R, tag="aw", name="amw")
            nc.sync.dma_start(amw[:], r(attnm_wT[:].rearrange("(k p) a -> p k a", p=P)))
            alw = const.tile([P, A // P], F32R)
            nc.sync.dma_start(alw[:], r(align_wT[:].rearrange("a p -> p a")))

            # keys -> tanh -> score partials; scores accumulated in SBUF rows
            sc_rows = [const.tile([1, S], F32, tag=f"scrow{b}", name=f"scrow{b}") for b in range(BSL)]
            for a in range(A // P):
                for bh in range(2):
                    kp = [ps_keys.tile([P, S], F32, tag="kp", name=f"kp{i}") for i in range(4)]
                    for k in range(NKC):
                        for i in range(4):
                            b = bh * 4 + i
                            nc.tensor.matmul(kp[i][:], r(amw[:, k, bass.ts(a, P)]),
                                             r(enc_s[:, k, b, :]),
                                             start=(k == 0), stop=(k == NKC - 1))
                    for i in range(4):
                        b = bh * 4 + i
                        tt = tanhp.tile([P, S], F32R, tag="tanh")
                        nc.scalar.activation(tt[:], kp[i][:], AF.Tanh,
                                             bias=queryT[:, a, b:b + 1])
                        stmp = ps_t_tile([1, S])
                        nc.tensor.matmul(stmp[:], r(alw[:, a:a + 1]), r(tt[:]),
                                         start=True, stop=True)
                        if a == 0:
                            nc.vector.tensor_copy(sc_rows[b][:], stmp[:])
                        else:
                            nc.vector.tensor_add(sc_rows[b][:], sc_rows[b][:], stmp[:])

            # ---------- softmax per batch row ----------
            ctxT = const.tile([P, NKC, BSL], F32R)
            scratch = scr.tile([P, S], F32)
            for b in range(BSL):
                sc = sc_rows[b]
                mx = scr.tile([1, 1], F32, tag="mx")
                nc.vector.tensor_reduce(mx[:], sc[:], axis=mybir.AxisListType.X,
                                        op=mybir.AluOpType.max)
                nmx = scr.tile([1, 1], F32, tag="nmx")
                nc.vector.tensor_scalar_mul(nmx[:], mx[:], -1.0)
                ex = scr.tile([1, S], F32, tag="ex")
                sm = scr.tile([1, 1], F32, tag="sm")
                nc.scalar.activation(ex[:], sc[:], AF.Exp, bias=nmx[:], accum_out=sm[:])
                rinv = scr.tile([1, 1], F32, tag="rinv")
                nc.vector.reciprocal(rinv[:], sm[:])
                at = scr.tile([1, S], F32R, tag=f"at{b % 2}")
                nc.vector.tensor_tensor(at[:], ex[:], rinv[:, :1].to_broadcast([1, S]),
                                        op=mybir.AluOpType.mult)
                nc.sync.dma_start(attn_o[b:b + 1, :], at[:].bitcast(F32))

                # broadcast attn row across partitions, then ctx via fused mult+reduce
                bc = ps_m_tile([P, S])
                nc.tensor.matmul(bc[:], r(ones[:1, :]), r(at[:]), start=True, stop=True)
                for k in range(NKC):
                    nc.vector.tensor_tensor(scratch[:], enc_s[:, k, b, :].bitcast(F32),
                                            bc[:], op=mybir.AluOpType.mult)
                    nc.vector.tensor_reduce(ctxT[:, k, b:b + 1], scratch[:],
                                            axis=mybir.AxisListType.X,
                                            op=mybir.AluOpType.add)

            # ---------- input_feed_new = [h1, ctx] @ lin_out_w.T + b ----------
            lob = small.tile([1, H], F32R)
            nc.sync.dma_start(lob[:], r(lin_out_b[:]))
            ifn = small.tile([BSL, H], F32)
            for n in range(2):
                pif = ps_m_tile([BSL, 512])
                for k in range(2 * NKC):
                    wt = wstream.tile([P, 512], F32R, tag="w", name="wt")
                    nc.sync.dma_start(wt[:], r(lin_out_wT[bass.ts(k, P), bass.ts(n, 512)]))
                    lhsT = h1T_b[:, k, :] if k < NKC else ctxT[:, k - NKC, :]
                    nc.tensor.matmul(pif[:], r(lhsT), r(wt[:]), start=(k == 0), stop=False)
                nc.tensor.matmul(pif[:], r(ones[:1, :BSL]), r(lob[:, bass.ts(n, 512)]),
                                 start=False, stop=True)
                nc.scalar.activation(ifn[:, bass.ts(n, 512)], pif[:], AF.Identity)
            nc.sync.dma_start(ifn_o[:], ifn[:])

            # if_new^T chunks -> DRAM -> AllGather
            ifT = small.tile([P, NKC, BSL], F32)
            for k in range(NKC):
                pt2 = ps_t_tile([P, BSL])
                nc.tensor.transpose(pt2[:], ifn[:, bass.ts(k, P)], ident[:BSL, :BSL])
                nc.vector.tensor_copy(ifT[:, k, :], pt2[:])
            ag2_in = dram.tile([NKC, P, BSL], F32)
            ag2_out = dram.tile([N_CORES, NKC, P, BSL], F32, addr_space="Shared")
            nc.sync.dma_start(ag2_in[:].rearrange("k p b -> p k b"), ifT[:])
            nc.gpsimd.collective_compute(
                "AllGather", mybir.AluOpType.bypass,
                replica_groups=[list(range(N_CORES))],
                ins=[ag2_in.opt()], outs=[ag2_out.opt()],
            )
            ifT_all = const.tile([P, NKC, N_CORES, BSL], F32R)  # [p, k, c, j]
            for k in range(NKC):
                nc.sync.dma_start(ifT_all[:, k, :, :],
                                  r(ag2_out[:, k, :, :].rearrange("c p b -> p c b")))

            # ---------- logits = if_new @ out_w_c^T + out_b_c ----------
            for v in range(VPAD // 512):
                obc = small.tile([1, 512], F32R, tag="obc", name="obc", bufs=2)
                nc.sync.dma_start(obc[:], r(out_b_c[:, bass.ts(v, 512)]))
                pl = ps_m_tile([B, 512])
                for k in range(NKC):
                    owt = owpool.tile([P, 512], F32R, tag="ow")
                    nc.sync.dma_start(owt[:], r(out_wT[bass.ts(k, P), bass.ts(v, 512)]))
                    nc.tensor.matmul(pl[:], r(ifT_all[:, k, :, :]), r(owt[:]),
                                     start=(k == 0), stop=False)
                nc.tensor.matmul(pl[:], r(ones[:1, :B]), r(obc[:]),
                                 start=False, stop=True)
                lt = scr.tile([B, 512], F32, tag="lt")
                nc.scalar.activation(lt[:], pl[:], AF.Identity)
                nc.sync.dma_start(logits_o[:, bass.ts(v, 512)], lt[:])

    nc.compile()
    return nc


_NC_CACHE = {}


def _get_nc():
    if "nc" not in _NC_CACHE:
        _NC_CACHE["nc"] = build()
    return _NC_CACHE["nc"]


def _prep_inputs(tokens, h0, c0, encoder_outputs, input_feed, emb_table,
                 W_ih, W_hh, b_ih, b_hh, attnm_w, attnq_w, attnq_b,
                 align_w, lin_out_w, lin_out_b, out_w, out_b):
    f = np.ascontiguousarray
    tok = np.asarray(tokens).astype(np.int64).reshape(B)
    embT = f(np.asarray(emb_table).astype(np.float32)[tok].T)
    xT = np.zeros((E + H, B), np.float32)
    xT[E:, :] = np.asarray(input_feed[0]).T
    h0T = f(np.asarray(h0[0]).T.astype(np.float32))
    bias = (np.asarray(b_ih) + np.asarray(b_hh)).astype(np.float32)
    aqT = f(np.asarray(attnq_w).T.astype(np.float32))
    amT = f(np.asarray(attnm_w).T.astype(np.float32))
    aqb = f(np.asarray(attnq_b).astype(np.float32).reshape(A // P, P))
    alT = f(np.asarray(align_w).astype(np.float32).reshape(A // P, P))
    loT = f(np.asarray(lin_out_w).T.astype(np.float32))
    lob = np.asarray(lin_out_b).astype(np.float32).reshape(1, H)
    enc = np.asarray(encoder_outputs).astype(np.float32)
    W_ih = np.asarray(W_ih).astype(np.float32)
    W_hh = np.asarray(W_hh).astype(np.float32)
    out_w = np.asarray(out_w).astype(np.float32)
    out_b = np.asarray(out_b).astype(np.float32)
    c0f = np.asarray(c0[0]).astype(np.float32)

    in_maps = []
    for c in range(N_CORES):
        rows = np.concatenate([np.arange(g * H + c * HSL, g * H + (c + 1) * HSL)
                               for g in range(4)])
        w_ihT = f(W_ih[rows].T)
        w_hhT = f(W_hh[rows].T)
        bias_c = f(bias[rows].reshape(1, 4 * HSL))
        encT = f(enc[:, c * BSL:(c + 1) * BSL, :].transpose(2, 1, 0))
        v0 = c * VP
        v1 = min(V, (c + 1) * VP)
        owT = np.zeros((H, VPAD), np.float32)
        owT[:, :v1 - v0] = out_w[v0:v1].T
        obc = np.zeros((1, VPAD), np.float32)
        obc[0, :v1 - v0] = out_b[v0:v1]
        in_maps.append({
            "embT": embT, "xT": xT, "h0T": h0T,
            "c0_c": f(c0f[:, c * HSL:(c + 1) * HSL]),
            "w_ihT": w_ihT, "w_hhT": w_hhT, "bias_c": bias_c,
            "attnq_wT": aqT, "attnq_b": aqb, "attnm_wT": amT,
            "align_wT": alT, "encT": encT,
            "lin_out_wT": loT, "lin_out_b": lob,
            "out_wT": owT, "out_b_c": obc,
        })
    return in_maps


def _assemble(results):
    logits = np.concatenate(
        [results[c]["logits_o"][:, :min(V, (c + 1) * VP) - c * VP] for c in range(N_CORES)],
        axis=1)[None]
    h1 = np.concatenate([results[c]["h1_o"] for c in range(N_CORES)], axis=1)[None]
    c1 = np.concatenate([results[c]["c1_o"] for c in range(N_CORES)], axis=1)[None]
    attn = np.concatenate([results[c]["attn_o"] for c in range(N_CORES)], axis=0).T
    ifn = np.concatenate([results[c]["ifn_o"] for c in range(N_CORES)], axis=0)[None]
    return (np.ascontiguousarray(logits), h1, c1, np.ascontiguousarray(attn), ifn)


def kernel(**inputs):
    nc = _get_nc()
    in_maps = _prep_inputs(**inputs)
    res = run_bass_kernel_spmd(nc, in_maps, list(range(N_CORES)), trace=False)
    return _assemble(res.results)
